# revision 18
# baseline (speedup 1.0000x reference)
"""3-layer edge-gated GCN (PyG GCNConv-style) on 8 TRN2 NeuronCores.

Strategy (self-contained, shapes hardcoded for N=50000, E=800000, D=256):
  - Shard nodes 8 ways (6250/core, padded to 6272 = 49*128 rows).
  - Algebra: with deg[v] = sum_{dst=v} ew + 1, dinv = deg^-1/2,
      h'   = (x @ W + b) * dinv[:, None]
      out  = relu?( dinv * (SUM_{e: dst=v} ew_e * h'[src_e]  +  h'[v]) )
    which equals the reference GCN layer exactly (dinv[src] folded into h',
    dinv[dst] folded into the epilogue, self-loop = dinv^2 * h).
  - Per layer: local matmul -> AllGather h' (bf16) -> per-edge row gather
    (dma_gather, int16 idx, two 25088-row tables) -> segment-sum via TensorE
    matmuls against on-device-built one-hot*ew matrices -> fused epilogue.
  - Edges are partitioned by dst owner, grouped per 128-dst tile, split into
    lo/hi source-table halves, padded to a uniform block count so all 8 cores
    run the identical program (SPMD).
"""
import os
import sys
sys.path.insert(0, "/opt/trn_rl_repo")

import numpy as np
import ml_dtypes

import concourse.bass as bass
import concourse.tile as tile
from concourse import bacc, mybir
from concourse.bass_utils import run_bass_kernel_spmd

F32 = mybir.dt.float32
BF16 = mybir.dt.bfloat16
I16 = mybir.dt.int16

N, E, D = 50000, 800000, 256
C = 8                 # cores
SH = N // C           # 6250 real rows per shard
T = 49                # dst tiles per core
SHP = T * 128         # 6272 padded rows per shard
NP = C * SHP          # 50176 padded global rows
HALF = NP // 2        # 25088 (= shards of cores 0-3) -> table A / table B
AG_SPLIT = 24         # tiles 0..23 -> first-half collective, 24..48 -> second


def _host_prep(x, edge_index, edge_attr):
    """Pure index/layout preprocessing (no float math on values)."""
    src = np.asarray(edge_index[0], dtype=np.int64)
    dst = np.asarray(edge_index[1], dtype=np.int64)
    attr = np.asarray(edge_attr, dtype=np.float32).reshape(-1)

    owner_d = dst // SH
    dl = dst - owner_d * SH              # 0..6249
    tl = dl // 128                       # dst tile 0..48
    dcol = dl % 128
    owner_s = src // SH
    gs = owner_s * SHP + (src - owner_s * SH)   # padded global src id
    hi = (gs >= HALF).astype(np.int64)

    # segment = (core, tile, half); stable order by segment
    seg = (owner_d * T + tl) * 2 + hi
    order = np.argsort(seg, kind="stable")
    seg_sorted = seg[order]
    counts = np.bincount(seg_sorted, minlength=C * T * 2)
    seg_starts = np.concatenate([[0], np.cumsum(counts)[:-1]])
    rank_in_seg = np.arange(E) - seg_starts[seg_sorted]

    n_lo = counts[0::2].reshape(C, T)
    n_hi = counts[1::2].reshape(C, T)
    B_lo = int(np.max((n_lo + 127) // 128))
    B_hi = int(np.max((n_hi + 127) // 128))
    B = B_lo + B_hi
    e_pad = T * B * 128                  # uniform padded edges per core

    # destination slot for each (sorted) edge inside its core's edge array
    t_sorted = (seg_sorted // 2) % T
    hi_sorted = seg_sorted % 2
    core_sorted = seg_sorted // (T * 2)
    slot = t_sorted * (B * 128) + hi_sorted * (B_lo * 128) + rank_in_seg

    gidx_all = np.zeros((C, e_pad), dtype=np.int64)       # default dummy -> 0
    dcol_all = np.full((C, e_pad), -1.0, dtype=np.float32)
    attr_all = np.zeros((C, e_pad), dtype=np.float32)

    eidx = order                                           # original edge ids
    g_sorted = gs[order]
    g_sorted = np.where(hi_sorted == 1, g_sorted - HALF, g_sorted)
    dcol_sorted = dcol[order].astype(np.float32)
    attr_sorted = attr[order]
    gidx_all[core_sorted, slot] = g_sorted
    dcol_all[core_sorted, slot] = dcol_sorted
    attr_all[core_sorted, slot] = attr_sorted

    # dma_gather idx layout: idx i of the whole array -> [16k + i%16, i//16]
    # (valid because every per-call slot base is a multiple of 16)
    i = np.arange(e_pad)
    idx_tiles = []
    for c in range(C):
        t16 = np.zeros((16, e_pad // 16), dtype=np.int16)
        t16[i % 16, i // 16] = gidx_all[c].astype(np.int16)
        idx_tiles.append(np.tile(t16, (8, 1)))             # replicate per Q7 core

    # token-major [128, e_pad/128]
    attr_tm = [attr_all[c].reshape(-1, 128).T.copy() for c in range(C)]
    # structural one-hot S01: [T, 128 edge-partition, B*128 dst-col] bf16
    Bn = B
    s01 = np.zeros((C, T, 128, Bn * 128), dtype=ml_dtypes.bfloat16)
    cc_i, sl_i = np.nonzero(dcol_all >= 0)
    t_i = sl_i // (Bn * 128)
    r_i = sl_i % 128
    b_i = (sl_i // 128) % Bn
    col_i = b_i * 128 + dcol_all[cc_i, sl_i].astype(np.int64)
    s01[cc_i, t_i, r_i, col_i] = 1.0
    s01_tiles = [np.ascontiguousarray(s01[c]) for c in range(C)]

    # x shards padded
    xs = []
    xf = np.asarray(x, dtype=np.float32)
    for c in range(C):
        pad = np.zeros((SHP, D), dtype=np.float32)
        pad[:SH] = xf[c * SH:(c + 1) * SH]
        xs.append(pad)
    return xs, idx_tiles, s01_tiles, attr_tm, B_lo, B_hi


def _build(B_lo, B_hi):
    B = B_lo + B_hi
    e_pad = T * B * 128
    NBLK = T * B

    nc = bacc.Bacc("TRN2", target_bir_lowering=False, debug=False,
                   num_devices=C, num_swdge_queues=4)

    x_d = nc.declare_dram_parameter("x", [SHP, D], F32, isOutput=False)
    idx_d = nc.declare_dram_parameter("idx", [128, e_pad // 16], I16, isOutput=False)
    s01_d = nc.declare_dram_parameter("s01", [T, 128, B * 128], BF16, isOutput=False)
    attr_d = nc.declare_dram_parameter("attr", [128, NBLK], F32, isOutput=False)
    W_d = [nc.declare_dram_parameter(f"W{l+1}", [D, D], F32, isOutput=False)
           for l in range(3)]
    b_d = [nc.declare_dram_parameter(f"b{l+1}", [128, D], F32, isOutput=False)
           for l in range(3)]
    mw1_d = nc.declare_dram_parameter("mw1", [128, 8], F32, isOutput=False)
    mb1_d = nc.declare_dram_parameter("mb1", [128, 8], F32, isOutput=False)
    mw2_d = nc.declare_dram_parameter("mw2", [128, 8], F32, isOutput=False)
    mb2_d = nc.declare_dram_parameter("mb2", [128, 1], F32, isOutput=False)
    ident_d = nc.declare_dram_parameter("ident", [128, 128], BF16, isOutput=False)
    out_d = nc.declare_dram_parameter("out", [128, T, D], F32, isOutput=True)

    s_ew_d = nc.dram_tensor("s_ew", [T, 128, B * 128], BF16)
    ag_in = [nc.dram_tensor(f"ag_in{l}", [SHP, D], BF16) for l in range(3)]
    ag_out = [nc.dram_tensor(f"ag_out{l}", [NP, D], BF16, addr_space="Shared")
              for l in range(3)]
    ag_st = [nc.dram_tensor(f"ag_st{l}", [NP, D], BF16) for l in range(2)]

    AL = mybir.AluOpType

    with tile.TileContext(nc) as tc:
        with (
            tc.tile_pool(name="res", bufs=1) as res,          # resident tiles
            tc.tile_pool(name="work", bufs=3) as work,
            tc.tile_pool(name="gath", bufs=4) as gath,
            tc.tile_pool(name="spool", bufs=8) as spool,
            tc.tile_pool(name="ppool", bufs=2, space="PSUM") as ppool,
            tc.tile_pool(name="ptr", bufs=2, space="PSUM") as ptr,
            tc.tile_pool(name="pagg", bufs=3, space="PSUM") as pagg,
        ):
            # ---- resident loads ----
            x_res = res.tile([128, T, D], BF16, tag="x_res")
            idx_r = res.tile([128, e_pad // 16], I16, tag="idx")
            ew_r = res.tile([128, NBLK], F32, tag="ew")
            hb16 = res.tile([128, T, D], BF16, tag="hb16")
            dinv_r = res.tile([128, T], F32, tag="dinv")
            ident_r = res.tile([128, 128], BF16, tag="ident")
            ones_r = res.tile([128, 1], BF16, tag="ones")
            Wt = [res.tile([128, 2, D], BF16, name=f"Wt{l}", tag=f"W{l}") for l in range(3)]
            bt = [res.tile([128, D], F32, name=f"bt{l}", tag=f"b{l}") for l in range(3)]
            mw1_r = res.tile([128, 8], F32, tag="mw1")
            mb1_r = res.tile([128, 8], F32, tag="mb1")
            mw2_r = res.tile([128, 8], F32, tag="mw2")
            mb2_r = res.tile([128, 1], F32, tag="mb2")

            nc.sync.dma_start(idx_r[:], idx_d.ap())
            nc.sync.dma_start(ident_r[:], ident_d.ap())
            nc.sync.dma_start(mw1_r[:], mw1_d.ap())
            nc.sync.dma_start(mb1_r[:], mb1_d.ap())
            nc.sync.dma_start(mw2_r[:], mw2_d.ap())
            nc.sync.dma_start(mb2_r[:], mb2_d.ap())
            for l in range(3):
                nc.gpsimd.dma_start(     # f32 -> bf16 cast during DMA
                    Wt[l][:], W_d[l].ap().rearrange("(k p) o -> p k o", p=128))
                nc.sync.dma_start(bt[l][:], b_d[l].ap())
            nc.gpsimd.dma_start(         # x cast to bf16, tiled layout
                x_res[:], x_d.ap().rearrange("(t p) d -> p t d", p=128))
            nc.gpsimd.memset(ones_r[:], 1.0)

            # ---- edge MLP: ew = sigmoid(relu(a*mw1+mb1) @ mw2 + mb2) ----
            attr_r = work.tile([128, NBLK], F32, tag="attr", bufs=1)
            nc.sync.dma_start(attr_r[:], attr_d.ap())
            acc = None
            for j in range(8):
                tj = work.tile([128, NBLK], F32, tag="mlptmp", bufs=2)
                nc.scalar.activation(tj[:], attr_r[:],
                                     mybir.ActivationFunctionType.Relu,
                                     bias=mb1_r[:, j:j + 1],
                                     scale=mw1_r[:, j:j + 1])
                nacc = work.tile([128, NBLK], F32, tag="mlpacc", bufs=2,
                                 name=f"acc{j}")
                if j == 0:
                    nc.vector.tensor_scalar_mul(nacc[:], tj[:], mw2_r[:, j:j + 1])
                else:
                    nc.vector.scalar_tensor_tensor(
                        nacc[:], tj[:], mw2_r[:, j:j + 1], acc[:],
                        op0=AL.mult, op1=AL.add)
                acc = nacc
            nc.scalar.activation(ew_r[:], acc[:],
                                 mybir.ActivationFunctionType.Sigmoid,
                                 bias=mb2_r[:, 0:1])

            # ---- degree pass + S_ew build: S_ew = S01*ew; deg = S_ew^T@1 + 1 ----
            for t in range(T):
                s01_t = gath.tile([128, B * 128], BF16, tag="sew", bufs=2)
                nc.sync.dma_start(s01_t[:], s01_d.ap()[t])
                sew_t = gath.tile([128, B * 128], BF16, tag="sewo", bufs=2)
                dp = ptr.tile([128, 1], F32, tag="degp", bufs=1)
                for b in range(B):
                    blk = t * B + b
                    nc.vector.tensor_scalar_mul(
                        sew_t[:, b * 128:(b + 1) * 128],
                        s01_t[:, b * 128:(b + 1) * 128], ew_r[:, blk:blk + 1])
                    nc.tensor.matmul(dp[:], sew_t[:, b * 128:(b + 1) * 128],
                                     ones_r[:],
                                     start=(b == 0), stop=(b == B - 1))
                nc.sync.dma_start(s_ew_d.ap()[t], sew_t[:])
                degs = work.tile([128, 1], F32, tag="degs")
                nc.vector.tensor_scalar_add(degs[:], dp[:], 1.0)
                rec = work.tile([128, 1], F32, tag="rec")
                nc.vector.reciprocal(rec[:], degs[:])
                nc.scalar.sqrt(dinv_r[:, t:t + 1], rec[:])

            KSTAGE = int(os.environ.get("KSTAGE", "0"))
            if KSTAGE == 7:
                # gathers only, straight from an input DRAM table
                tbl7 = s01_d.ap().rearrange("t p c -> (t p) c")[:, 0:256]
                reg7 = nc.gpsimd.to_reg(1152)
                for t in range(T):
                    base16 = t * 18 * 8
                    g7a = gath.tile([128, 9, 2304 // 9], BF16, tag="g7a", bufs=6,
                                    name=f"g7a_{t}")
                    g7b = gath.tile([128, 9, 2304 // 9], BF16, tag="g7b", bufs=6,
                                    name=f"g7b_{t}")
                    nc.gpsimd.dma_gather(
                        g7a[:], tbl7, idx_r[:, base16:base16 + 72],
                        num_idxs=1152, num_idxs_reg=reg7, elem_size=D,
                        elem_step=2304, single_packet=False, queue_num=(2 * t) % 4)
                    nc.gpsimd.dma_gather(
                        g7b[:], tbl7, idx_r[:, base16 + 72:base16 + 144],
                        num_idxs=1152, num_idxs_reg=reg7, elem_size=D,
                        elem_step=2304, single_packet=False, queue_num=(2 * t + 1) % 4)
                    ob7 = work.tile([128, D], F32, tag="outb", bufs=3,
                                    name=f"ob7_{t}")
                    nc.vector.tensor_add(ob7[:], g7a[:, 0, :], g7b[:, 0, :])
                    nc.sync.dma_start(out_d.ap()[:, t, :], ob7[:])
                layers = []
            elif KSTAGE in (5, 6):
                layers = [0]
            elif KSTAGE == 1:
                nc.sync.dma_start(out_d.ap()[:, :, 0], dinv_r[:])
                layers = []
            elif KSTAGE in (2, 3):
                layers = [0]
            elif KSTAGE == 4:
                layers = [0, 1]
            else:
                layers = [0, 1, 2]
            last = layers[-1] if layers else -1
            reg_lo = nc.gpsimd.to_reg(B_lo * 128)
            reg_hi = nc.gpsimd.to_reg(B_hi * 128)
            agA = [ag_st[l % 2].ap()[0:HALF] for l in range(3)]
            agB = [ag_st[l % 2].ap()[HALF:NP] for l in range(3)]

            for l in layers:
                # ---- phase A: h' = (x @ W + b) * dinv ----
                for t in range(T):
                    xt = x_res[:, t, :]
                    tp = ptr.tile([128, 2, 128], BF16, tag="tpsum")
                    nc.tensor.transpose(tp[:, 0, :], xt[:, 0:128], ident_r[:])
                    nc.tensor.transpose(tp[:, 1, :], xt[:, 128:256], ident_r[:])
                    xT = work.tile([128, 2, 128], BF16, tag="xT")
                    nc.vector.tensor_copy(xT[:, 0, :], tp[:, 0, :])
                    nc.vector.tensor_copy(xT[:, 1, :], tp[:, 1, :])
                    hp = ppool.tile([128, D], F32, tag="hpsum")
                    nc.tensor.matmul(hp[:], xT[:, 0, :], Wt[l][:, 0, :],
                                     start=True, stop=False)
                    nc.tensor.matmul(hp[:], xT[:, 1, :], Wt[l][:, 1, :],
                                     start=False, stop=True)
                    tmp = work.tile([128, D], F32, tag="phA")
                    nc.vector.tensor_add(tmp[:], hp[:], bt[l][:])
                    nc.vector.tensor_scalar_mul(hb16[:, t, :], tmp[:],
                                                dinv_r[:, t:t + 1])
                # bounce to internal DRAM, then AllGather
                nc.sync.dma_start(
                    ag_in[l].ap().rearrange("(t p) d -> p t d", p=128),
                    hb16[:])
                nc.gpsimd.collective_compute(
                    "AllGather", AL.bypass,
                    replica_groups=[list(range(C))],
                    ins=[ag_in[l].ap().opt()],
                    outs=[ag_out[l].ap().opt()],
                )
                stg = ag_st[l % 2]
                nc.gpsimd.dma_start(
                    stg.ap().rearrange("(p n) d -> p (n d)", p=128),
                    ag_out[l].ap().rearrange("(p n) d -> p (n d)", p=128))

                if KSTAGE == 2:
                    nc.gpsimd.dma_start(out_d.ap(), hb16[:])
                    continue
                if KSTAGE == 5:
                    # gathers only; consume via DVE copy to out
                    for t in range(T):
                        base16 = t * B * 8
                        glo = gath.tile([128, B_lo, D], BF16, tag="glo")
                        ghi = gath.tile([128, B_hi, D], BF16, tag="ghi")
                        nc.gpsimd.dma_gather(
                            glo[:], agA[l], idx_r[:, base16:base16 + B_lo * 8],
                            num_idxs=B_lo * 128, num_idxs_reg=reg_lo, elem_size=D,
                            single_packet=False)
                        nc.gpsimd.dma_gather(
                            ghi[:], agB[l], idx_r[:, base16 + B_lo * 8:base16 + B * 8],
                            num_idxs=B_hi * 128, num_idxs_reg=reg_hi, elem_size=D,
                            single_packet=False)
                        ob5 = work.tile([128, D], F32, tag="outb", bufs=3)
                        nc.vector.tensor_add(ob5[:], glo[:, 0, :], ghi[:, 0, :])
                        nc.sync.dma_start(out_d.ap()[:, t, :], ob5[:])
                    continue
                # ---- phase B: gather + segment matmul + epilogue ----
                # process tiles in pairs: 4 gathers on 4 queues back-to-back
                for t0 in range(0, T, 2):
                    tiles = [t for t in (t0, t0 + 1) if t < T]
                    gs = {}
                    sews = {}
                    for j, t in enumerate(tiles):
                        base16 = t * B * 8
                        sew_t = gath.tile([128, B * 128], BF16, tag="sewL",
                                          bufs=3, name=f"sew_{l}_{t}")
                        nc.sync.dma_start(sew_t[:], s_ew_d.ap()[t])
                        sews[t] = sew_t
                        glo = gath.tile([128, B_lo, D], BF16, tag="glo",
                                        bufs=6, name=f"glo_{l}_{t}")
                        ghi = gath.tile([128, B_hi, D], BF16, tag="ghi",
                                        bufs=6, name=f"ghi_{l}_{t}")
                        nc.gpsimd.dma_gather(
                            glo[:], agA[l], idx_r[:, base16:base16 + B_lo * 8],
                            num_idxs=B_lo * 128, num_idxs_reg=reg_lo, elem_size=D,
                            single_packet=False, queue_num=(2 * j) % 4)
                        nc.gpsimd.dma_gather(
                            ghi[:], agB[l], idx_r[:, base16 + B_lo * 8:base16 + B * 8],
                            num_idxs=B_hi * 128, num_idxs_reg=reg_hi, elem_size=D,
                            single_packet=False, queue_num=(2 * j + 1) % 4)
                        gs[t] = (glo, ghi)
                    for t in tiles:
                        glo, ghi = gs[t]
                        sew_t = sews[t]
                        ap_ = pagg.tile([128, D], F32, tag="aggp", name=f"ap_{l}_{t}")
                        for b in range(B):
                            g_ap = glo[:, b, :] if b < B_lo else ghi[:, b - B_lo, :]
                            nc.tensor.matmul(ap_[:], sew_t[:, b * 128:(b + 1) * 128],
                                             g_ap,
                                             start=(b == 0), stop=(b == B - 1))
                        tmp = work.tile([128, D], F32, tag="phB", name=f"tmp_{l}_{t}")
                        nc.vector.tensor_add(tmp[:], ap_[:], hb16[:, t, :])
                        if l == last:
                            ob = work.tile([128, D], F32, tag="outb", bufs=3,
                                           name=f"ob_{t}")
                            nc.vector.tensor_scalar_mul(
                                ob[:], tmp[:], dinv_r[:, t:t + 1])
                            nc.sync.dma_start(out_d.ap()[:, t, :], ob[:])
                        else:
                            nc.vector.tensor_scalar(
                                x_res[:, t, :], tmp[:], dinv_r[:, t:t + 1], 0.0,
                                op0=AL.mult, op1=AL.max)


    nc.compile()
    return nc


_CACHE = {}


def kernel(x, edge_index, edge_attr, W1, b1, W2, b2, W3, b3, mw1, mb1, mw2, mb2):
    xs, idx_tiles, s01_tiles, attr_tm, B_lo, B_hi = _host_prep(x, edge_index, edge_attr)
    if os.environ.get("KSTAGE") == "7":
        idx_tiles = [np.minimum(it, 6271).astype(np.int16) for it in idx_tiles]

    key = (B_lo, B_hi)
    if key not in _CACHE:
        _CACHE[key] = _build(B_lo, B_hi)
    nc = _CACHE[key]

    ident = np.eye(128, dtype=np.float32).astype(ml_dtypes.bfloat16)
    b_bc = [np.tile(np.asarray(b, np.float32)[None, :], (128, 1))
            for b in (b1, b2, b3)]
    mw1_b = np.tile(np.asarray(mw1, np.float32).reshape(1, 8), (128, 1))
    mb1_b = np.tile(np.asarray(mb1, np.float32).reshape(1, 8), (128, 1))
    mw2_b = np.tile(np.asarray(mw2, np.float32).reshape(1, 8), (128, 1))
    mb2_b = np.tile(np.asarray(mb2, np.float32).reshape(1, 1), (128, 1))
    Ws = [np.ascontiguousarray(np.asarray(w, np.float32)) for w in (W1, W2, W3)]

    in_maps = []
    for c in range(C):
        in_maps.append({
            "x": xs[c], "idx": idx_tiles[c], "s01": s01_tiles[c],
            "attr": attr_tm[c],
            "W1": Ws[0], "W2": Ws[1], "W3": Ws[2],
            "b1": b_bc[0], "b2": b_bc[1], "b3": b_bc[2],
            "mw1": mw1_b, "mb1": mb1_b, "mw2": mw2_b, "mb2": mb2_b,
            "ident": ident,
        })
    res = run_bass_kernel_spmd(nc, in_maps, core_ids=list(range(C)))
    outs = []
    for c in range(C):
        o = res.results[c]["out"]            # [128, T, D]
        rows = o.transpose(1, 0, 2).reshape(SHP, D)[:SH]
        outs.append(rows)
    return np.concatenate(outs, axis=0).astype(np.float32)


# revision 19
# speedup vs baseline: 4.1009x; 4.1009x over previous
"""3-layer edge-gated GCN (PyG GCNConv-style) on 8 TRN2 NeuronCores.

Strategy (self-contained, shapes hardcoded for N=50000, E=800000, D=256):
  - Shard nodes 8 ways (6250/core, padded to 6272 = 49*128 rows).
  - Algebra: with deg[v] = sum_{dst=v} ew + 1, dinv = deg^-1/2,
      h'   = (x @ W + b) * dinv[:, None]
      out  = relu?( dinv * (SUM_{e: dst=v} ew_e * h'[src_e]  +  h'[v]) )
    which equals the reference GCN layer exactly (dinv[src] folded into h',
    dinv[dst] folded into the epilogue, self-loop = dinv^2 * h).
  - Per layer: local matmul -> AllGather h' (bf16) -> per-edge row gather
    (dma_gather, int16 idx, two 25088-row tables) -> segment-sum via TensorE
    matmuls against on-device-built one-hot*ew matrices -> fused epilogue.
  - Edges are partitioned by dst owner, grouped per 128-dst tile, split into
    lo/hi source-table halves, padded to a uniform block count so all 8 cores
    run the identical program (SPMD).
"""
import os
import sys
sys.path.insert(0, "/opt/trn_rl_repo")

import numpy as np
import ml_dtypes

import concourse.bass as bass
import concourse.tile as tile
from concourse import bacc, mybir
from concourse.bass_utils import run_bass_kernel_spmd

F32 = mybir.dt.float32
BF16 = mybir.dt.bfloat16
I16 = mybir.dt.int16

N, E, D = 50000, 800000, 256
C = 8                 # cores
SH = N // C           # 6250 real rows per shard
T = 49                # dst tiles per core
SHP = T * 128         # 6272 padded rows per shard
NP = C * SHP          # 50176 padded global rows
HALF = NP // 2        # 25088 (= shards of cores 0-3) -> table A / table B
AG_SPLIT = 24         # tiles 0..23 -> first-half collective, 24..48 -> second


def _host_prep(x, edge_index, edge_attr):
    """Pure index/layout preprocessing (no float math on values)."""
    src = np.asarray(edge_index[0], dtype=np.int64)
    dst = np.asarray(edge_index[1], dtype=np.int64)
    attr = np.asarray(edge_attr, dtype=np.float32).reshape(-1)

    owner_d = dst // SH
    dl = dst - owner_d * SH              # 0..6249
    tl = dl // 128                       # dst tile 0..48
    dcol = dl % 128
    owner_s = src // SH
    gs = owner_s * SHP + (src - owner_s * SH)   # padded global src id
    hi = (gs >= HALF).astype(np.int64)

    # segment = (core, tile, half); stable order by segment
    seg = (owner_d * T + tl) * 2 + hi
    order = np.argsort(seg, kind="stable")
    seg_sorted = seg[order]
    counts = np.bincount(seg_sorted, minlength=C * T * 2)
    seg_starts = np.concatenate([[0], np.cumsum(counts)[:-1]])
    rank_in_seg = np.arange(E) - seg_starts[seg_sorted]

    n_lo = counts[0::2].reshape(C, T)
    n_hi = counts[1::2].reshape(C, T)
    B_lo = int(np.max((n_lo + 127) // 128))
    B_hi = int(np.max((n_hi + 127) // 128))
    B = B_lo + B_hi
    e_pad = T * B * 128                  # uniform padded edges per core

    # destination slot for each (sorted) edge inside its core's edge array
    t_sorted = (seg_sorted // 2) % T
    hi_sorted = seg_sorted % 2
    core_sorted = seg_sorted // (T * 2)
    slot = t_sorted * (B * 128) + hi_sorted * (B_lo * 128) + rank_in_seg

    gidx_all = np.zeros((C, e_pad), dtype=np.int64)       # default dummy -> 0
    dcol_all = np.full((C, e_pad), -1.0, dtype=np.float32)
    attr_all = np.zeros((C, e_pad), dtype=np.float32)

    eidx = order                                           # original edge ids
    g_sorted = gs[order]
    g_sorted = np.where(hi_sorted == 1, g_sorted - HALF, g_sorted)
    dcol_sorted = dcol[order].astype(np.float32)
    attr_sorted = attr[order]
    gidx_all[core_sorted, slot] = g_sorted
    dcol_all[core_sorted, slot] = dcol_sorted
    attr_all[core_sorted, slot] = attr_sorted

    # dma_gather idx layout: idx i of the whole array -> [16k + i%16, i//16]
    # (valid because every per-call slot base is a multiple of 16)
    i = np.arange(e_pad)
    idx_tiles = []
    for c in range(C):
        t16 = np.zeros((16, e_pad // 16), dtype=np.int16)
        t16[i % 16, i // 16] = gidx_all[c].astype(np.int16)
        idx_tiles.append(np.tile(t16, (8, 1)))             # replicate per Q7 core

    # token-major [128, e_pad/128]
    attr_tm = [attr_all[c].reshape(-1, 128).T.copy() for c in range(C)]
    # structural one-hot S01: [T, 128 edge-partition, B*128 dst-col] bf16
    Bn = B
    s01 = np.zeros((C, T, 128, Bn * 128), dtype=ml_dtypes.bfloat16)
    cc_i, sl_i = np.nonzero(dcol_all >= 0)
    t_i = sl_i // (Bn * 128)
    r_i = sl_i % 128
    b_i = (sl_i // 128) % Bn
    col_i = b_i * 128 + dcol_all[cc_i, sl_i].astype(np.int64)
    s01[cc_i, t_i, r_i, col_i] = 1.0
    s01_tiles = [np.ascontiguousarray(s01[c]) for c in range(C)]

    # x shards padded
    xs = []
    xf = np.asarray(x, dtype=np.float32)
    for c in range(C):
        pad = np.zeros((SHP, D), dtype=np.float32)
        pad[:SH] = xf[c * SH:(c + 1) * SH]
        xs.append(pad)
    return xs, idx_tiles, s01_tiles, attr_tm, B_lo, B_hi


def _build(B_lo, B_hi):
    B = B_lo + B_hi
    e_pad = T * B * 128
    NBLK = T * B

    nc = bacc.Bacc("TRN2", target_bir_lowering=False, debug=False,
                   num_devices=C, num_swdge_queues=4)

    x_d = nc.declare_dram_parameter("x", [SHP, D], F32, isOutput=False)
    idx_d = nc.declare_dram_parameter("idx", [128, e_pad // 16], I16, isOutput=False)
    s01_d = nc.declare_dram_parameter("s01", [T, 128, B * 128], BF16, isOutput=False)
    attr_d = nc.declare_dram_parameter("attr", [128, NBLK], F32, isOutput=False)
    W_d = [nc.declare_dram_parameter(f"W{l+1}", [D, D], F32, isOutput=False)
           for l in range(3)]
    b_d = [nc.declare_dram_parameter(f"b{l+1}", [128, D], F32, isOutput=False)
           for l in range(3)]
    mw1_d = nc.declare_dram_parameter("mw1", [128, 8], F32, isOutput=False)
    mb1_d = nc.declare_dram_parameter("mb1", [128, 8], F32, isOutput=False)
    mw2_d = nc.declare_dram_parameter("mw2", [128, 8], F32, isOutput=False)
    mb2_d = nc.declare_dram_parameter("mb2", [128, 1], F32, isOutput=False)
    ident_d = nc.declare_dram_parameter("ident", [128, 128], BF16, isOutput=False)
    out_d = nc.declare_dram_parameter("out", [128, T, D], F32, isOutput=True)

    s_ew_d = nc.dram_tensor("s_ew", [T, 128, B * 128], BF16)
    ag_in = [nc.dram_tensor(f"ag_in{l}", [SHP, D], BF16) for l in range(3)]
    ag_out = [nc.dram_tensor(f"ag_out{l}", [NP, D], BF16, addr_space="Shared")
              for l in range(3)]
    ag_st = [nc.dram_tensor(f"ag_st{l}", [NP, D], BF16) for l in range(2)]

    AL = mybir.AluOpType

    with tile.TileContext(nc) as tc:
        with (
            tc.tile_pool(name="res", bufs=1) as res,          # resident tiles
            tc.tile_pool(name="work", bufs=3) as work,
            tc.tile_pool(name="gath", bufs=4) as gath,
            tc.tile_pool(name="spool", bufs=8) as spool,
            tc.tile_pool(name="ppool", bufs=2, space="PSUM") as ppool,
            tc.tile_pool(name="ptr", bufs=2, space="PSUM") as ptr,
            tc.tile_pool(name="pagg", bufs=3, space="PSUM") as pagg,
        ):
            # ---- resident loads ----
            x_res = res.tile([128, T, D], BF16, tag="x_res")
            idx_r = res.tile([128, e_pad // 16], I16, tag="idx")
            ew_r = res.tile([128, NBLK], F32, tag="ew")
            hb16 = res.tile([128, T, D], BF16, tag="hb16")
            dinv_r = res.tile([128, T], F32, tag="dinv")
            ident_r = res.tile([128, 128], BF16, tag="ident")
            ones_r = res.tile([128, 1], BF16, tag="ones")
            Wt = [res.tile([128, 2, D], BF16, name=f"Wt{l}", tag=f"W{l}") for l in range(3)]
            bt = [res.tile([128, D], F32, name=f"bt{l}", tag=f"b{l}") for l in range(3)]
            mw1_r = res.tile([128, 8], F32, tag="mw1")
            mb1_r = res.tile([128, 8], F32, tag="mb1")
            mw2_r = res.tile([128, 8], F32, tag="mw2")
            mb2_r = res.tile([128, 1], F32, tag="mb2")

            nc.sync.dma_start(idx_r[:], idx_d.ap())
            nc.sync.dma_start(ident_r[:], ident_d.ap())
            nc.sync.dma_start(mw1_r[:], mw1_d.ap())
            nc.sync.dma_start(mb1_r[:], mb1_d.ap())
            nc.sync.dma_start(mw2_r[:], mw2_d.ap())
            nc.sync.dma_start(mb2_r[:], mb2_d.ap())
            for l in range(3):
                nc.gpsimd.dma_start(     # f32 -> bf16 cast during DMA
                    Wt[l][:], W_d[l].ap().rearrange("(k p) o -> p k o", p=128))
                nc.sync.dma_start(bt[l][:], b_d[l].ap())
            nc.gpsimd.dma_start(         # x cast to bf16, tiled layout
                x_res[:], x_d.ap().rearrange("(t p) d -> p t d", p=128))
            nc.gpsimd.memset(ones_r[:], 1.0)

            # ---- edge MLP: ew = sigmoid(relu(a*mw1+mb1) @ mw2 + mb2) ----
            attr_r = work.tile([128, NBLK], F32, tag="attr", bufs=1)
            nc.sync.dma_start(attr_r[:], attr_d.ap())
            acc = None
            for j in range(8):
                tj = work.tile([128, NBLK], F32, tag="mlptmp", bufs=2)
                nc.scalar.activation(tj[:], attr_r[:],
                                     mybir.ActivationFunctionType.Relu,
                                     bias=mb1_r[:, j:j + 1],
                                     scale=mw1_r[:, j:j + 1])
                nacc = work.tile([128, NBLK], F32, tag="mlpacc", bufs=2,
                                 name=f"acc{j}")
                if j == 0:
                    nc.vector.tensor_scalar_mul(nacc[:], tj[:], mw2_r[:, j:j + 1])
                else:
                    nc.vector.scalar_tensor_tensor(
                        nacc[:], tj[:], mw2_r[:, j:j + 1], acc[:],
                        op0=AL.mult, op1=AL.add)
                acc = nacc
            nc.scalar.activation(ew_r[:], acc[:],
                                 mybir.ActivationFunctionType.Sigmoid,
                                 bias=mb2_r[:, 0:1])

            # ---- degree pass + S_ew build: S_ew = S01*ew; deg = S_ew^T@1 + 1 ----
            for t in range(T):
                s01_t = gath.tile([128, B * 128], BF16, tag="sew", bufs=2)
                nc.sync.dma_start(s01_t[:], s01_d.ap()[t])
                sew_t = gath.tile([128, B * 128], BF16, tag="sewo", bufs=2)
                dp = ptr.tile([128, 1], F32, tag="degp", bufs=1)
                for b in range(B):
                    blk = t * B + b
                    nc.vector.tensor_scalar_mul(
                        sew_t[:, b * 128:(b + 1) * 128],
                        s01_t[:, b * 128:(b + 1) * 128], ew_r[:, blk:blk + 1])
                    nc.tensor.matmul(dp[:], sew_t[:, b * 128:(b + 1) * 128],
                                     ones_r[:],
                                     start=(b == 0), stop=(b == B - 1))
                nc.sync.dma_start(s_ew_d.ap()[t], sew_t[:])
                degs = work.tile([128, 1], F32, tag="degs")
                nc.vector.tensor_scalar_add(degs[:], dp[:], 1.0)
                rec = work.tile([128, 1], F32, tag="rec")
                nc.vector.reciprocal(rec[:], degs[:])
                nc.scalar.sqrt(dinv_r[:, t:t + 1], rec[:])

            KSTAGE = int(os.environ.get("KSTAGE", "0"))
            if KSTAGE == 8:
                # stage-7 gathers + real PE matmul consumers (no collective)
                tbl8 = s01_d.ap().rearrange("t p c -> (t p) c")[:, 0:256]
                reg8 = nc.gpsimd.to_reg(1152)
                for t in range(T):
                    base16 = t * 18 * 8
                    g8a = gath.tile([128, 9, D], BF16, tag="glo", bufs=6,
                                    name=f"g8a_{t}")
                    g8b = gath.tile([128, 9, D], BF16, tag="ghi", bufs=6,
                                    name=f"g8b_{t}")
                    sew8 = gath.tile([128, B * 128], BF16, tag="sewL", bufs=3,
                                     name=f"sew8_{t}")
                    nc.sync.dma_start(sew8[:], s_ew_d.ap()[t])
                    nc.gpsimd.dma_gather(
                        g8a[:], tbl8, idx_r[:, base16:base16 + 72],
                        num_idxs=1152, num_idxs_reg=reg8, elem_size=D,
                        elem_step=2304, single_packet=False, queue_num=(2 * t) % 4)
                    nc.gpsimd.dma_gather(
                        g8b[:], tbl8, idx_r[:, base16 + 72:base16 + 144],
                        num_idxs=1152, num_idxs_reg=reg8, elem_size=D,
                        elem_step=2304, single_packet=False, queue_num=(2 * t + 1) % 4)
                    ap8 = pagg.tile([128, D], F32, tag="aggp", name=f"ap8_{t}")
                    for b in range(B):
                        g_ap = g8a[:, b, :] if b < 9 else g8b[:, b - 9, :]
                        nc.tensor.matmul(ap8[:], sew8[:, b * 128:(b + 1) * 128],
                                         g_ap, start=(b == 0), stop=(b == B - 1))
                    ob8 = work.tile([128, D], F32, tag="outb", bufs=3,
                                    name=f"ob8_{t}")
                    nc.vector.tensor_scalar_mul(ob8[:], ap8[:], dinv_r[:, t:t + 1])
                    nc.sync.dma_start(out_d.ap()[:, t, :], ob8[:])
                layers = []
            elif KSTAGE == 7:
                # gathers only, straight from an input DRAM table
                tbl7 = s01_d.ap().rearrange("t p c -> (t p) c")[:, 0:256]
                reg7 = nc.gpsimd.to_reg(1152)
                for t in range(T):
                    base16 = t * 18 * 8
                    g7a = gath.tile([128, 9, 2304 // 9], BF16, tag="g7a", bufs=6,
                                    name=f"g7a_{t}")
                    g7b = gath.tile([128, 9, 2304 // 9], BF16, tag="g7b", bufs=6,
                                    name=f"g7b_{t}")
                    nc.gpsimd.dma_gather(
                        g7a[:], tbl7, idx_r[:, base16:base16 + 72],
                        num_idxs=1152, num_idxs_reg=reg7, elem_size=D,
                        elem_step=2304, single_packet=False, queue_num=(2 * t) % 4)
                    nc.gpsimd.dma_gather(
                        g7b[:], tbl7, idx_r[:, base16 + 72:base16 + 144],
                        num_idxs=1152, num_idxs_reg=reg7, elem_size=D,
                        elem_step=2304, single_packet=False, queue_num=(2 * t + 1) % 4)
                    ob7 = work.tile([128, D], F32, tag="outb", bufs=3,
                                    name=f"ob7_{t}")
                    nc.vector.tensor_add(ob7[:], g7a[:, 0, :], g7b[:, 0, :])
                    nc.sync.dma_start(out_d.ap()[:, t, :], ob7[:])
                layers = []
            elif KSTAGE in (5, 6):
                layers = [0]
            elif KSTAGE == 1:
                nc.sync.dma_start(out_d.ap()[:, :, 0], dinv_r[:])
                layers = []
            elif KSTAGE in (2, 3):
                layers = [0]
            elif KSTAGE == 4:
                layers = [0, 1]
            else:
                layers = [0, 1, 2]
            last = layers[-1] if layers else -1
            reg_lo = nc.gpsimd.to_reg(B_lo * 128)
            reg_hi = nc.gpsimd.to_reg(B_hi * 128)
            agA = [ag_st[l % 2].ap()[0:HALF] for l in range(3)]
            agB = [ag_st[l % 2].ap()[HALF:NP] for l in range(3)]

            for l in layers:
                # ---- phase A: h' = (x @ W + b) * dinv ----
                for t in range(T):
                    xt = x_res[:, t, :]
                    tp = ptr.tile([128, 2, 128], BF16, tag="tpsum")
                    nc.tensor.transpose(tp[:, 0, :], xt[:, 0:128], ident_r[:])
                    nc.tensor.transpose(tp[:, 1, :], xt[:, 128:256], ident_r[:])
                    xT = work.tile([128, 2, 128], BF16, tag="xT")
                    nc.vector.tensor_copy(xT[:, 0, :], tp[:, 0, :])
                    nc.vector.tensor_copy(xT[:, 1, :], tp[:, 1, :])
                    hp = ppool.tile([128, D], F32, tag="hpsum")
                    nc.tensor.matmul(hp[:], xT[:, 0, :], Wt[l][:, 0, :],
                                     start=True, stop=False)
                    nc.tensor.matmul(hp[:], xT[:, 1, :], Wt[l][:, 1, :],
                                     start=False, stop=True)
                    tmp = work.tile([128, D], F32, tag="phA")
                    nc.vector.tensor_add(tmp[:], hp[:], bt[l][:])
                    nc.vector.tensor_scalar_mul(hb16[:, t, :], tmp[:],
                                                dinv_r[:, t:t + 1])
                # bounce to internal DRAM, then AllGather
                nc.sync.dma_start(
                    ag_in[l].ap().rearrange("(t p) d -> p t d", p=128),
                    hb16[:])
                nc.gpsimd.collective_compute(
                    "AllGather", AL.bypass,
                    replica_groups=[list(range(C))],
                    ins=[ag_in[l].ap().opt()],
                    outs=[ag_out[l].ap().opt()],
                )
                stg = ag_st[l % 2]
                nc.gpsimd.dma_start(
                    stg.ap().rearrange("(p n) d -> p (n d)", p=128),
                    ag_out[l].ap().rearrange("(p n) d -> p (n d)", p=128))

                if KSTAGE == 2:
                    nc.gpsimd.dma_start(out_d.ap(), hb16[:])
                    continue
                if KSTAGE == 5:
                    # gathers only; consume via DVE copy to out
                    for t in range(T):
                        base16 = t * B * 8
                        glo = gath.tile([128, B_lo, D], BF16, tag="glo")
                        ghi = gath.tile([128, B_hi, D], BF16, tag="ghi")
                        nc.gpsimd.dma_gather(
                            glo[:], agA[l], idx_r[:, base16:base16 + B_lo * 8],
                            num_idxs=B_lo * 128, num_idxs_reg=reg_lo, elem_size=D,
                            single_packet=False)
                        nc.gpsimd.dma_gather(
                            ghi[:], agB[l], idx_r[:, base16 + B_lo * 8:base16 + B * 8],
                            num_idxs=B_hi * 128, num_idxs_reg=reg_hi, elem_size=D,
                            single_packet=False)
                        ob5 = work.tile([128, D], F32, tag="outb", bufs=3)
                        nc.vector.tensor_add(ob5[:], glo[:, 0, :], ghi[:, 0, :])
                        nc.sync.dma_start(out_d.ap()[:, t, :], ob5[:])
                    continue
                # ---- phase B: gather + segment matmul + epilogue ----
                # process tiles in pairs: 4 gathers on 4 queues back-to-back
                for t0 in range(0, T, 2):
                    tiles = [t for t in (t0, t0 + 1) if t < T]
                    gs = {}
                    sews = {}
                    for j, t in enumerate(tiles):
                        base16 = t * B * 8
                        sew_t = gath.tile([128, B * 128], BF16, tag="sewL",
                                          bufs=3, name=f"sew_{l}_{t}")
                        nc.sync.dma_start(sew_t[:], s_ew_d.ap()[t])
                        sews[t] = sew_t
                        glo = gath.tile([128, B_lo, D], BF16, tag="glo",
                                        bufs=6, name=f"glo_{l}_{t}")
                        ghi = gath.tile([128, B_hi, D], BF16, tag="ghi",
                                        bufs=6, name=f"ghi_{l}_{t}")
                        nc.gpsimd.dma_gather(
                            glo[:], agA[l], idx_r[:, base16:base16 + B_lo * 8],
                            num_idxs=B_lo * 128, num_idxs_reg=reg_lo, elem_size=D,
                            single_packet=False, queue_num=(2 * j) % 4)
                        nc.gpsimd.dma_gather(
                            ghi[:], agB[l], idx_r[:, base16 + B_lo * 8:base16 + B * 8],
                            num_idxs=B_hi * 128, num_idxs_reg=reg_hi, elem_size=D,
                            single_packet=False, queue_num=(2 * j + 1) % 4)
                        gs[t] = (glo, ghi)
                    for t in tiles:
                        glo, ghi = gs[t]
                        sew_t = sews[t]
                        ap_ = pagg.tile([128, D], F32, tag="aggp", name=f"ap_{l}_{t}")
                        for b in range(B):
                            g_ap = glo[:, b, :] if b < B_lo else ghi[:, b - B_lo, :]
                            nc.tensor.matmul(ap_[:], sew_t[:, b * 128:(b + 1) * 128],
                                             g_ap,
                                             start=(b == 0), stop=(b == B - 1))
                        tmp = work.tile([128, D], F32, tag="phB", name=f"tmp_{l}_{t}")
                        nc.vector.tensor_add(tmp[:], ap_[:], hb16[:, t, :])
                        if l == last:
                            ob = work.tile([128, D], F32, tag="outb", bufs=3,
                                           name=f"ob_{t}")
                            nc.vector.tensor_scalar_mul(
                                ob[:], tmp[:], dinv_r[:, t:t + 1])
                            nc.sync.dma_start(out_d.ap()[:, t, :], ob[:])
                        else:
                            nc.vector.tensor_scalar(
                                x_res[:, t, :], tmp[:], dinv_r[:, t:t + 1], 0.0,
                                op0=AL.mult, op1=AL.max)


    nc.compile()
    return nc


_CACHE = {}


def kernel(x, edge_index, edge_attr, W1, b1, W2, b2, W3, b3, mw1, mb1, mw2, mb2):
    xs, idx_tiles, s01_tiles, attr_tm, B_lo, B_hi = _host_prep(x, edge_index, edge_attr)
    if os.environ.get("KSTAGE") == "7":
        idx_tiles = [np.minimum(it, 6271).astype(np.int16) for it in idx_tiles]

    key = (B_lo, B_hi)
    if key not in _CACHE:
        _CACHE[key] = _build(B_lo, B_hi)
    nc = _CACHE[key]

    ident = np.eye(128, dtype=np.float32).astype(ml_dtypes.bfloat16)
    b_bc = [np.tile(np.asarray(b, np.float32)[None, :], (128, 1))
            for b in (b1, b2, b3)]
    mw1_b = np.tile(np.asarray(mw1, np.float32).reshape(1, 8), (128, 1))
    mb1_b = np.tile(np.asarray(mb1, np.float32).reshape(1, 8), (128, 1))
    mw2_b = np.tile(np.asarray(mw2, np.float32).reshape(1, 8), (128, 1))
    mb2_b = np.tile(np.asarray(mb2, np.float32).reshape(1, 1), (128, 1))
    Ws = [np.ascontiguousarray(np.asarray(w, np.float32)) for w in (W1, W2, W3)]

    in_maps = []
    for c in range(C):
        in_maps.append({
            "x": xs[c], "idx": idx_tiles[c], "s01": s01_tiles[c],
            "attr": attr_tm[c],
            "W1": Ws[0], "W2": Ws[1], "W3": Ws[2],
            "b1": b_bc[0], "b2": b_bc[1], "b3": b_bc[2],
            "mw1": mw1_b, "mb1": mb1_b, "mw2": mw2_b, "mb2": mb2_b,
            "ident": ident,
        })
    res = run_bass_kernel_spmd(nc, in_maps, core_ids=list(range(C)))
    outs = []
    for c in range(C):
        o = res.results[c]["out"]            # [128, T, D]
        rows = o.transpose(1, 0, 2).reshape(SHP, D)[:SH]
        outs.append(rows)
    return np.concatenate(outs, axis=0).astype(np.float32)
